# revision 10
# baseline (speedup 1.0000x reference)
"""Tensor-parallel causal MHA kernel for 8 Trainium2 NeuronCores (v2c).

Same algorithm as v2a, but Phase A (transposes + QKV projections) for slab
s+1 is interleaved into Phase B's (attention) ACT-bound stream for q-slab s,
at key-block granularity. Causality makes this legal: attention for q-slab s
only needs Q/K/V up to slab s. Per-slab qt/kt/vt tiles give the scheduler
precise dependencies; a single shared 2-slot PSUM pool serves transposes,
projections, broadcasts and the O-projection so all PSUM fits in 8 banks.
"""

import sys

if "/opt/trn_rl_repo" not in sys.path:
    sys.path.insert(0, "/opt/trn_rl_repo")

import numpy as np

import concourse.bass as bass
import concourse.tile as tile
from concourse import bacc, mybir
from concourse.bass_utils import run_bass_kernel_spmd
from concourse.masks import make_identity

F32 = mybir.dt.float32
F32R = mybir.dt.float32r
BF16 = mybir.dt.bfloat16
EXP = mybir.ActivationFunctionType.Exp
IDENT = mybir.ActivationFunctionType.Identity

N_CORES = 8


def build_program(L=2048, D=1024, HPC=8, hd=64, repeat=1, with_bias=False,
                  diag_mode="select", narrow=True, burst=0):
    DQ = HPC * hd
    SL = 512
    NS = L // SL
    TSUB = SL // 128
    DIN = D // 128
    DQT = DQ // 128
    NP = HPC // 2
    VW = hd + 1

    nc = bacc.Bacc("TRN2", target_bir_lowering=False, debug=False)

    x_d = nc.dram_tensor("x", [L, D], F32R, kind="ExternalInput")
    wq_d = nc.dram_tensor("wq", [D, DQ], BF16, kind="ExternalInput")
    wk_d = nc.dram_tensor("wk", [D, DQ], BF16, kind="ExternalInput")
    wv_d = nc.dram_tensor("wv", [D, DQ], BF16, kind="ExternalInput")
    wo_d = nc.dram_tensor("wo", [DQ, D], BF16, kind="ExternalInput")
    if with_bias:
        bq_d = nc.dram_tensor("bq", [DQ], F32, kind="ExternalInput")  # pre-scaled
        bk_d = nc.dram_tensor("bk", [DQ], F32, kind="ExternalInput")
        bv_d = nc.dram_tensor("bv", [1, DQ], BF16, kind="ExternalInput")
    out_d = nc.dram_tensor("out", [L, D], F32, kind="ExternalOutput")

    with tile.TileContext(nc) as tc:
        with (
            tc.tile_pool(name="persist", bufs=1) as persist,
            tc.tile_pool(name="consts", bufs=1) as consts,
        ):
            # per-slab persistent activations (precise scheduler deps)
            qts = [persist.tile([128, DQT, SL], BF16, tag=f"qt{s}", name=f"qt{s}")
                   for s in range(NS)]
            kts = [persist.tile([128, DQT, SL], BF16, tag=f"kt{s}", name=f"kt{s}")
                   for s in range(NS)]
            vts = [persist.tile([128, TSUB, HPC, VW], BF16, tag=f"vt{s}",
                                name=f"vt{s}")
                   for s in range(NS)]

            ident = consts.tile([128, 128], F32, tag="ident")
            ident_r = consts.tile([128, 128], F32R, tag="ident_r")
            ones_b = consts.tile([1, hd], BF16, tag="ones_b")
            make_identity(nc, ident[:])
            # f32r operands must come from f32r-rounding producers (ACT) or DMA
            nc.scalar.activation(
                ident_r[:], ident[:], mybir.ActivationFunctionType.Copy
            )
            nc.gpsimd.memset(ones_b[:], 1.0)
            if diag_mode == "maskmm":
                idr_b = consts.tile([128, 128], BF16, tag="idr_b")
                maskc = consts.tile([128, TSUB, SL], BF16, tag="maskc")
                make_identity(nc, idr_b[:])
                nc.gpsimd.memset(maskc[:], 0.0)
                for j in range(TSUB):
                    nc.gpsimd.affine_select(
                        out=maskc[:, j, :], in_=maskc[:, j, :],
                        compare_op=mybir.AluOpType.is_ge, fill=-1e30,
                        base=-128 * j, channel_multiplier=-1,
                        pattern=[[1, SL]],
                    )
            for s in range(NS):
                nc.gpsimd.memset(vts[s][:, :, :, hd], 1.0)
            if with_bias:
                ones_k = consts.tile([1, 128], BF16, tag="ones_k")
                bq_sb = consts.tile([128, DQT], F32, tag="bq")
                bk_sb = consts.tile([128, DQT], F32, tag="bk")
                bv_sb = consts.tile([1, DQ], BF16, tag="bv")
                nc.gpsimd.memset(ones_k[:], 1.0)
                nc.sync.dma_start(bq_sb[:], bq_d[:].rearrange("(c p) -> p c", p=128))
                nc.sync.dma_start(bk_sb[:], bk_d[:].rearrange("(c p) -> p c", p=128))
                nc.sync.dma_start(bv_sb[:], bv_d[:])

            def one_pass():
              with (
                tc.tile_pool(name="w", bufs=1) as wpool,
                tc.tile_pool(name="xa", bufs=8) as xa_pool,
                tc.tile_pool(name="xt", bufs=2) as xt_pool,
                tc.tile_pool(name="att", bufs=6) as att_pool,
                tc.tile_pool(name="attoT", bufs=2) as attoT_pool,
                tc.tile_pool(name="nrm", bufs=4) as nrm_pool,
                tc.tile_pool(name="outsb", bufs=4) as out_pool,
                tc.tile_pool(name="pwork", bufs=2, space="PSUM") as pw_pool,
                tc.tile_pool(name="pscore", bufs=2, space="PSUM") as pscore_pool,
                tc.tile_pool(name="pav", bufs=2, space="PSUM") as pav_pool,
              ):
                wq_sb = wpool.tile([128, DIN, DQ], BF16, tag="wq")
                wk_sb = wpool.tile([128, DIN, DQ], BF16, tag="wk")
                wv_sb = wpool.tile([128, DIN, DQ], BF16, tag="wv")
                wo_sb = wpool.tile([128, DQT, D], BF16, tag="wo")
                nc.sync.dma_start(
                    wq_sb[:], wq_d[:].rearrange("(c p) d -> p c d", p=128)
                )
                nc.sync.dma_start(
                    wk_sb[:], wk_d[:].rearrange("(c p) d -> p c d", p=128)
                )
                nc.sync.dma_start(
                    wv_sb[:], wv_d[:].rearrange("(c p) d -> p c d", p=128)
                )
                nc.sync.dma_start(
                    wo_sb[:], wo_d[:].rearrange("(c p) d -> p c d", p=128)
                )

                def emit_xa(s):
                    xa = []
                    for ts in range(TSUB):
                        t = xa_pool.tile([128, D], F32R, tag="xa",
                                         name=f"xa{s}_{ts}")
                        nc.sync.dma_start(
                            t[:],
                            x_d[s * SL + ts * 128 : s * SL + (ts + 1) * 128, :],
                        )
                        xa.append(t)
                    return xa

                def a_units(s, xa):
                    """Phase A for slab s as a list of PE work-unit closures."""
                    xt = xt_pool.tile([128, DIN, SL], BF16, tag="xt",
                                      name=f"xt{s}")
                    units = []

                    def mk_t(dc):
                        def go():
                            pxt = pw_pool.tile([128, SL], F32, tag="pw",
                                               name=f"pxt{s}_{dc}")
                            for ts in range(TSUB):
                                nc.tensor.transpose(
                                    pxt[:, ts * 128 : (ts + 1) * 128].bitcast(F32R),
                                    xa[ts][:, dc * 128 : (dc + 1) * 128],
                                    ident_r[:],
                                )
                            nc.vector.tensor_copy(xt[:, dc, :], pxt[:])
                        return go

                    def mk_qk(w_sb, dst, bname, i):
                        def go():
                            pq = pw_pool.tile([128, SL], F32, tag="pw",
                                              name=f"pp{s}_{bname}{i}")
                            for dc in range(DIN):
                                nc.tensor.matmul(
                                    pq[:],
                                    w_sb[:, dc, i * 128 : (i + 1) * 128],
                                    xt[:, dc, :],
                                    start=(dc == 0),
                                    stop=(dc == DIN - 1),
                                )
                            dslice = dst[:, i, :]
                            if with_bias:
                                bias = (bq_sb if bname == "q" else bk_sb)[:, i : i + 1]
                                nc.scalar.activation(dslice, pq[:], IDENT, bias=bias)
                            else:
                                nc.vector.tensor_copy(dslice, pq[:])
                        return go

                    def mk_v(ts):
                        def go():
                            pv = pw_pool.tile([128, DQ], F32, tag="pw",
                                              name=f"pv{s}_{ts}")
                            for dc in range(DIN):
                                nc.tensor.matmul(
                                    pv[:],
                                    xt[:, dc, ts * 128 : (ts + 1) * 128],
                                    wv_sb[:, dc, :],
                                    start=(dc == 0),
                                    stop=(dc == DIN - 1) and not with_bias,
                                )
                            if with_bias:
                                nc.tensor.matmul(
                                    pv[:], ones_k[:], bv_sb[:],
                                    start=False, stop=True,
                                )
                            nc.vector.tensor_copy(
                                vts[s][:, ts, :, 0:hd],
                                pv[:].rearrange("p (h c) -> p h c", c=hd),
                            )
                        return go

                    for dc in range(DIN):
                        units.append(mk_t(dc))
                    # interleave q/k/v so consumers come online evenly
                    for i in range(DQT):
                        units.append(mk_qk(wq_sb, qts[s], "q", i))
                        units.append(mk_qk(wk_sb, kts[s], "k", i))
                    for ts in range(TSUB):
                        units.append(mk_v(ts))
                    return units

                # deferred-op queue: AV/norm/O-proj trail the score/exp stream
                stream = []

                def flush(keep, limit=0):
                    n = 0
                    while len(stream) > keep:
                        stream.pop(0)()
                        n += 1
                        if limit and n >= limit:
                            break

                # Phase A for slab 0 runs un-interleaved (nothing to overlap)
                xa0 = emit_xa(0)
                for u in a_units(0, xa0):
                    u()

                for qs in range(NS):
                    # stage next slab's x loads + A work-units for injection
                    au = []
                    if qs + 1 < NS:
                        xa_n = emit_xa(qs + 1)
                        au = a_units(qs + 1, xa_n)
                    attoT = attoT_pool.tile([128, DQT, SL], BF16, tag="attoT",
                                            name=f"attoT{qs}")
                    nkb = (qs + 1) * TSUB
                    steps = NP * nkb
                    stride = max(1, steps // len(au)) if au else 0
                    step = 0
                    for p in range(NP):
                        pavs = [
                            pav_pool.tile([VW, SL], F32, tag="pav", name=f"pav{j}")
                            for j in range(2)
                        ]
                        for kb in range(nkb):
                            sb, kk = kb // TSUB, kb % TSUB
                            jd = kb - qs * TSUB
                            n0 = 128 * jd if (jd >= 0 and narrow) else 0
                            ps = pscore_pool.tile([128, 2, SL], F32, tag="ps")
                            domask = jd >= 0 and diag_mode == "maskmm"
                            for j in range(2):
                                nc.tensor.matmul(
                                    ps[:, j, n0:SL],
                                    kts[sb][j * hd : (j + 1) * hd, p,
                                            kk * 128 : (kk + 1) * 128],
                                    qts[qs][j * hd : (j + 1) * hd, p, n0:SL],
                                    start=True,
                                    stop=not domask,
                                )
                                if domask:
                                    nc.tensor.matmul(
                                        ps[:, j, n0:SL],
                                        idr_b[:],
                                        maskc[:, jd, n0:SL],
                                        start=False,
                                        stop=True,
                                    )
                            att = att_pool.tile([128, 2, SL], BF16, tag="att")
                            nc.scalar.activation(
                                att[:, :, n0:SL], ps[:, :, n0:SL], EXP
                            )
                            if jd >= 0 and diag_mode == "select":
                                for j in range(2):
                                    nc.gpsimd.affine_select(
                                        out=att[:, j, n0:SL],
                                        in_=att[:, j, n0:SL],
                                        compare_op=mybir.AluOpType.is_ge,
                                        fill=0.0,
                                        base=0,
                                        channel_multiplier=-1,
                                        pattern=[[1, SL - n0]],
                                    )

                            def mk_av(kb=kb, sb=sb, kk=kk, n0=n0, att=att,
                                      pavs=pavs, p=p,
                                      first=(kb == 0), last=(kb == nkb - 1)):
                                def go():
                                    for j in range(2):
                                        nc.tensor.matmul(
                                            pavs[j][:, n0:SL],
                                            vts[sb][:, kk, 2 * p + j, :],
                                            att[:, j, n0:SL],
                                            start=first,
                                            stop=last,
                                        )
                                return go

                            stream.append(mk_av())
                            flush(3, limit=burst)
                            step += 1
                            if au and stride and step % stride == 0:
                                au.pop(0)()

                        def mk_norm(p=p, pavs=pavs, attoT=attoT):
                            def go():
                                for j in range(2):
                                    recip = nrm_pool.tile([1, SL], BF16,
                                                          tag="recip")
                                    with nc.allow_low_precision("softmax recip"):
                                        nc.vector.reciprocal(
                                            recip[:], pavs[j][hd : hd + 1, :]
                                        )
                                    pbc = pw_pool.tile([hd, SL], F32, tag="pw",
                                                       name="pbc")
                                    nc.tensor.matmul(
                                        pbc[:], ones_b[:], recip[:],
                                        start=True, stop=True,
                                    )
                                    bc = nrm_pool.tile([hd, SL], F32, tag="bc")
                                    nc.vector.tensor_copy(bc[:], pbc[:])
                                    nc.vector.tensor_mul(
                                        attoT[j * hd : (j + 1) * hd, p, :],
                                        pavs[j][0:hd, :],
                                        bc[:],
                                    )
                            return go

                        stream.append(mk_norm())
                        flush(3)

                    # leftover A units for the next slab
                    while au:
                        au.pop(0)()

                    # O projection for this slab (partial over the head shard)
                    for ts in range(TSUB):
                        for ob in range(D // SL):
                            def mk_o(qs=qs, ts=ts, ob=ob, attoT=attoT):
                                def go():
                                    po = pw_pool.tile([128, SL], F32, tag="pw",
                                                      name="po")
                                    for c in range(DQT):
                                        nc.tensor.matmul(
                                            po[:],
                                            attoT[:, c, ts * 128 : (ts + 1) * 128],
                                            wo_sb[:, c, ob * SL : (ob + 1) * SL],
                                            start=(c == 0),
                                            stop=(c == DQT - 1),
                                        )
                                    osb = out_pool.tile([128, SL], F32, tag="osb")
                                    nc.vector.tensor_copy(osb[:], po[:])
                                    nc.sync.dma_start(
                                        out_d[
                                            qs * SL + ts * 128 :
                                            qs * SL + (ts + 1) * 128,
                                            ob * SL : (ob + 1) * SL,
                                        ],
                                        osb[:],
                                    )
                                return go

                            stream.append(mk_o())
                            flush(12)

                flush(0)

            for _rep in range(repeat):
                one_pass()

    nc.compile()
    return nc


_PROGRAMS = {}


def _get_program(with_bias=False):
    if with_bias not in _PROGRAMS:
        _PROGRAMS[with_bias] = build_program(with_bias=with_bias)
    return _PROGRAMS[with_bias]


def _bf16(a: np.ndarray) -> np.ndarray:
    return np.ascontiguousarray(a, dtype=np.float32).astype(mybir.dt.np(BF16))


def _shard_inputs(x, Wq, bq, Wk, bk, Wv, bv, Wo, bo, with_bias=False):
    """Build the 8 per-core input maps."""
    HIDDEN = Wq.shape[0]
    M = 2
    DQ = HIDDEN // M
    hd = 64
    s = np.float32(1.0 / np.sqrt(hd))
    in_maps = []
    for c in range(N_CORES):
        b = c // M
        g = c % M
        cols = slice(g * DQ, (g + 1) * DQ)
        m = {
            "x": np.ascontiguousarray(x[b]),
            "wq": _bf16(Wq[:, cols] * s),
            "wk": _bf16(Wk[:, cols]),
            "wv": _bf16(Wv[:, cols]),
            "wo": _bf16(Wo[cols, :]),
        }
        if with_bias:
            m["bq"] = np.ascontiguousarray(bq[cols] * s)
            m["bk"] = np.ascontiguousarray(bk[cols])
            m["bv"] = _bf16(bv[cols])[None, :]
        in_maps.append(m)
    return in_maps


def kernel(**inputs) -> np.ndarray:
    x = np.asarray(inputs["x"], dtype=np.float32)
    B, L, D = x.shape
    with_bias = any(
        np.any(np.asarray(inputs[k])) for k in ("bq", "bk", "bv")
    )
    nc = _get_program(with_bias)
    in_maps = _shard_inputs(
        x,
        np.asarray(inputs["Wq"], np.float32), np.asarray(inputs["bq"], np.float32),
        np.asarray(inputs["Wk"], np.float32), np.asarray(inputs["bk"], np.float32),
        np.asarray(inputs["Wv"], np.float32), np.asarray(inputs["bv"], np.float32),
        np.asarray(inputs["Wo"], np.float32), np.asarray(inputs["bo"], np.float32),
        with_bias=with_bias,
    )
    res = run_bass_kernel_spmd(nc, in_maps, list(range(N_CORES)))
    bo = np.asarray(inputs["bo"], np.float32)
    out = np.empty((B, L, D), np.float32)
    for b in range(B):
        out[b] = res.results[2 * b]["out"] + res.results[2 * b + 1]["out"] + bo
    return out
